# revision 3
# baseline (speedup 1.0000x reference)
"""3-layer GAT (graph attention) on 8 trn2 NeuronCores.

Strategy: shard destination nodes across cores (6250/core, padded to
6272 rows). Per layer: each core computes its shard of an augmented
table  [h | es | ed | 1 | pad]  (h = y@W, es/ed = per-node attention
terms) via TensorE, AllGathers the 50176x132 table, then processes its
edges (sorted by dst, padded to a uniform subtile schedule): indirect-
DMA gathers h-rows by src, a one-hot(dst)*exp(logit) matrix is built in
one VectorE op, and a TensorE matmul accumulates both the weighted
feature sum and the softmax denominator (ones column) per 128-dst tile.
"""
import sys
sys.path.insert(0, "/opt/trn_rl_repo")
import numpy as np
import concourse.bass as bass
import concourse.mybir as mybir
import concourse.tile as tile
from concourse.bass_utils import run_bass_kernel_spmd

N = 50000
E = 600000
F_IN, HID, F_OUT = 64, 128, 64
NC_ = 8
SH = 6250          # real nodes per shard
SHP = 6272         # padded shard rows (49 * 128)
NT = NC_ * SHP     # table rows
NTILE = SHP // 128 # 49 dst tiles per core
TW = 132           # table row width: h(128), es, ed, one, pad
NEG_ATT = 0.2
NEG_ACT = 0.01
DUMMY = 6250       # global table row used by padding edges (core0 pad row)

f32 = mybir.dt.float32
i32 = mybir.dt.int32


def _hoist_waits(nc):
    """walrus on this toolchain allows only ONE sync-wait slot per
    instruction; move extras onto preceding same-engine NoOps."""
    for fn in nc.m.functions:
        for blk in fn.blocks:
            new_insts = []
            for inst in blk.instructions:
                si = getattr(inst, "sync_info", None)
                waits = list(si.on_wait) if si is not None and si.on_wait else []
                if len(waits) > 1:
                    keep, extra = waits[:1], waits[1:]
                    while extra:
                        chunk, extra = extra[:1], extra[1:]
                        nop = mybir.InstNoOp(
                            name=nc.get_next_instruction_name(),
                            engine=inst.engine, bass_nofuse=True)
                        nop.sync_info = mybir.SyncInfo(on_wait=chunk, on_update=[])
                        new_insts.append(nop)
                    inst.sync_info = mybir.SyncInfo(
                        on_wait=keep,
                        on_update=list(si.on_update) if si.on_update else [])
                new_insts.append(inst)
            blk.instructions = new_insts


def _build(smax):
    nsub = NTILE * smax
    nc = bass.Bass()
    xT = nc.declare_dram_parameter("xT", [F_IN, SHP], f32, isOutput=False)
    osrc = nc.declare_dram_parameter("osrc", [128, nsub], i32, isOutput=False)
    odst = nc.declare_dram_parameter("odst", [128, nsub], i32, isOutput=False)
    dloc = nc.declare_dram_parameter("dloc", [128, nsub], f32, isOutput=False)
    waug = [nc.declare_dram_parameter(f"waug{l}", [128, 130], f32, isOutput=False)
            for l in range(3)]
    wout = nc.declare_dram_parameter("wout", [128, F_OUT], f32, isOutput=False)
    bb = [nc.declare_dram_parameter(f"bb{l}", [128, 128], f32, isOutput=False)
          for l in range(3)]
    bbo = nc.declare_dram_parameter("bbo", [128, F_OUT], f32, isOutput=False)
    iota = nc.declare_dram_parameter("iota", [128, 128], f32, isOutput=False)
    ident = nc.declare_dram_parameter("ident", [128, 128], f32, isOutput=False)
    padneg = nc.declare_dram_parameter("padneg", [128, 1], f32, isOutput=False)
    out_e = nc.declare_dram_parameter("out", [SHP, F_OUT], f32, isOutput=True)

    tbl_in = [nc.dram_tensor(f"tbl_in{l}", [SHP, TW], f32) for l in range(3)]
    tbl = [nc.dram_tensor(f"tbl{l}", [NT, TW], f32, addr_space="Shared")
           for l in range(3)]

    with tile.TileContext(nc) as tc:
        with (
            tc.tile_pool(name="const", bufs=1) as cpool,
            tc.tile_pool(name="yt", bufs=1) as ypool,
            tc.tile_pool(name="stage", bufs=4) as spool,
            tc.tile_pool(name="gat", bufs=2 * smax + 4) as gpool,
            tc.tile_pool(name="ecol", bufs=3) as epool,
            tc.tile_pool(name="bx", bufs=4) as bxpool,
            tc.tile_pool(name="epi", bufs=4) as tpool,
            tc.tile_pool(name="ps_h", bufs=2, space="PSUM") as ps_h,
            tc.tile_pool(name="ps_nm", bufs=2, space="PSUM") as ps_nm,
            tc.tile_pool(name="ps_t", bufs=2, space="PSUM") as ps_t,
        ):
            # ---- resident constants / inputs
            osrc_sb = cpool.tile([128, nsub], i32)
            odst_sb = cpool.tile([128, nsub], i32)
            dloc_sb = cpool.tile([128, nsub], f32)
            nc.sync.dma_start(out=osrc_sb[:], in_=osrc[:])
            nc.sync.dma_start(out=odst_sb[:], in_=odst[:])
            nc.sync.dma_start(out=dloc_sb[:], in_=dloc[:])
            waug_sb = [cpool.tile([128, 130], f32, name=f"waug_sb{l}") for l in range(3)]
            for l in range(3):
                nc.sync.dma_start(out=waug_sb[l][:], in_=waug[l][:])
            wout_sb = cpool.tile([128, F_OUT], f32)
            nc.sync.dma_start(out=wout_sb[:], in_=wout[:])
            bb_sb = [cpool.tile([128, 128], f32, name=f"bb_sb{l}") for l in range(3)]
            for l in range(3):
                nc.sync.dma_start(out=bb_sb[l][:], in_=bb[l][:])
            bbo_sb = cpool.tile([128, F_OUT], f32)
            nc.sync.dma_start(out=bbo_sb[:], in_=bbo[:])
            iota_sb = cpool.tile([128, 128], f32)
            nc.sync.dma_start(out=iota_sb[:], in_=iota[:])
            id_sb = cpool.tile([128, 128], f32)
            nc.sync.dma_start(out=id_sb[:], in_=ident[:])
            pn_sb = cpool.tile([128, 1], f32)
            nc.sync.dma_start(out=pn_sb[:], in_=padneg[:])

            # YT double buffer: layer input, feat x nodes (feat on partitions)
            yt_sb = [cpool.tile([128, SHP], f32, name=f"yt{i}") for i in range(2)]
            nc.sync.dma_start(out=yt_sb[0][:F_IN, :], in_=xT[:])

            for l in range(3):
                fin = F_IN if l == 0 else HID
                ytc = yt_sb[l % 2]
                ytn = yt_sb[(l + 1) % 2]

                # ---- phase 1: shard of augmented table = yT' @ [W|ws|wd]
                for t in range(NTILE):
                    hps = ps_h.tile([128, 130], f32, tag="hps")
                    nc.tensor.matmul(hps[:], lhsT=ytc[:fin, t*128:(t+1)*128],
                                     rhs=waug_sb[l][:fin, :], start=True, stop=True)
                    stg = spool.tile([128, TW], f32, tag="stg")
                    nc.vector.memset(stg[:, 130:132], 0.0)
                    nc.vector.tensor_scalar_add(stg[:, 130:131], stg[:, 130:131], 1.0)
                    nc.vector.tensor_copy(out=stg[:, 0:130], in_=hps[:])
                    if t == NTILE - 1:
                        nc.vector.tensor_scalar_add(stg[:, 128:129],
                                                    stg[:, 128:129], pn_sb[:, :1])
                    nc.sync.dma_start(out=tbl_in[l][t*128:(t+1)*128, :], in_=stg[:])

                # ---- phase 2: allgather table
                nc.gpsimd.collective_compute(
                    "AllGather", mybir.AluOpType.bypass,
                    replica_groups=[list(range(NC_))],
                    ins=[tbl_in[l][:]], outs=[tbl[l][:]])

                # ---- phase 3: edges, per dst tile
                for d in range(NTILE):
                    esb = epool.tile([128, smax], f32, tag="esb")
                    edb = epool.tile([128, smax], f32, tag="edb")
                    exb = epool.tile([128, smax], f32, tag="exb")
                    gts = []
                    for s in range(smax):
                        j = d * smax + s
                        g = gpool.tile([128, TW], f32, tag="G")
                        nc.gpsimd.indirect_dma_start(
                            out=g[:], out_offset=None, in_=tbl[l][:],
                            in_offset=bass.IndirectOffsetOnAxis(
                                ap=osrc_sb[:, j:j+1], axis=0))
                        nc.gpsimd.indirect_dma_start(
                            out=edb[:, s:s+1], out_offset=None, in_=tbl[l][:],
                            in_offset=bass.IndirectOffsetOnAxis(
                                ap=odst_sb[:, j:j+1], axis=0),
                            element_offset=129)
                        nc.vector.tensor_copy(out=esb[:, s:s+1], in_=g[:, 128:129])
                        gts.append(g)
                    # logits -> exp, batched over the dst tile's subtiles
                    nc.vector.tensor_tensor(out=esb[:], in0=esb[:], in1=edb[:],
                                            op=mybir.AluOpType.add)
                    nc.vector.tensor_scalar_mul(edb[:], esb[:], NEG_ATT)
                    nc.vector.tensor_tensor(out=esb[:], in0=esb[:], in1=edb[:],
                                            op=mybir.AluOpType.max)
                    nc.scalar.activation(exb[:], esb[:],
                                         mybir.ActivationFunctionType.Exp)
                    nmps = ps_nm.tile([128, 131], f32, tag="nm")
                    for s in range(smax):
                        j = d * smax + s
                        bx = bxpool.tile([128, 128], f32, tag="bx")
                        nc.vector.tensor_scalar(
                            out=bx[:], in0=iota_sb[:],
                            scalar1=dloc_sb[:, j:j+1], scalar2=exb[:, s:s+1],
                            op0=mybir.AluOpType.is_equal,
                            op1=mybir.AluOpType.mult)
                        nc.tensor.matmul(nmps[:], lhsT=bx[:], rhs=gts[s][:, 0:131],
                                         start=(s == 0), stop=(s == smax - 1))
                    # ---- epilogue: y = numer/denom + b, activation, transpose
                    dn = tpool.tile([128, 1], f32, tag="dn")
                    nc.vector.tensor_scalar_add(dn[:], nmps[:, 130:131], 1e-16)
                    rec = tpool.tile([128, 1], f32, tag="rec")
                    nc.vector.reciprocal(rec[:], dn[:])
                    y = tpool.tile([128, 128], f32, tag="y")
                    nc.vector.tensor_scalar(
                        out=y[:], in0=nmps[:, 0:128], scalar1=rec[:],
                        scalar2=None, op0=mybir.AluOpType.mult)
                    nc.vector.tensor_tensor(out=y[:], in0=y[:], in1=bb_sb[l][:],
                                            op=mybir.AluOpType.add)
                    y2 = tpool.tile([128, 128], f32, tag="y2")
                    nc.vector.tensor_scalar_mul(y2[:], y[:], NEG_ACT)
                    nc.vector.tensor_tensor(out=y[:], in0=y[:], in1=y2[:],
                                            op=mybir.AluOpType.max)
                    tps = ps_t.tile([128, 128], f32, tag="tps")
                    nc.tensor.transpose(tps[:], y[:], id_sb[:])
                    if l < 2:
                        nc.vector.tensor_copy(out=ytn[:, d*128:(d+1)*128], in_=tps[:])
                    else:
                        y3t = tpool.tile([128, 128], f32, tag="y3t")
                        nc.vector.tensor_copy(out=y3t[:], in_=tps[:])
                        ops = ps_t.tile([128, F_OUT], f32, tag="ops")
                        nc.tensor.matmul(ops[:], lhsT=y3t[:], rhs=wout_sb[:],
                                         start=True, stop=True)
                        ot = tpool.tile([128, F_OUT], f32, tag="ot")
                        nc.vector.tensor_tensor(out=ot[:], in0=ops[:], in1=bbo_sb[:],
                                                op=mybir.AluOpType.add)
                        nc.sync.dma_start(out=out_e[d*128:(d+1)*128, :], in_=ot[:])
    _hoist_waits(nc)
    return nc


_CACHE = {}
LAST = None  # last BassKernelResults (for test harness introspection)


def _prep(x, edge_index, W0, as0, ad0, b0, W1, as1, ad1, b1,
          W2, as2, ad2, b2, Wout, bout):
    x = np.asarray(x, np.float32)
    ei = np.asarray(edge_index)
    src = np.concatenate([ei[0], np.arange(N, dtype=np.int64)]).astype(np.int64)
    dst = np.concatenate([ei[1], np.arange(N, dtype=np.int64)]).astype(np.int64)

    core = dst // SH
    row_of_src = (src // SH) * SHP + (src % SH)   # global table row of src

    per_core = []
    smax = 0
    for r in range(NC_):
        m = core == r
        s_r = row_of_src[m]
        d_r = dst[m] - r * SH           # local dst 0..6249
        o = np.argsort(d_r, kind="stable")
        s_r, d_r = s_r[o], d_r[o]
        t_r = d_r // 128                # dst tile
        cnt = np.bincount(t_r, minlength=NTILE)
        smax = max(smax, int(np.ceil(cnt.max() / 128)))
        per_core.append((s_r, d_r, t_r, cnt))

    nsub = NTILE * smax
    in_maps = []
    row_of_dst_all = []
    for r in range(NC_):
        s_r, d_r, t_r, cnt = per_core[r]
        osrc = np.full((128, nsub), DUMMY, np.int32)
        odst = np.full((128, nsub), DUMMY, np.int32)
        dloc = np.zeros((128, nsub), np.float32)
        start = np.zeros(NTILE, np.int64)
        start[1:] = np.cumsum(cnt)[:-1]
        k = np.arange(len(d_r)) - start[t_r]       # rank within dst tile
        sub = t_r * smax + (k // 128)              # subtile slot
        lane = k % 128
        osrc[lane, sub] = s_r
        odst[lane, sub] = (d_r // SH) * 0 + r * SHP + d_r  # row of dst in table
        dloc[lane, sub] = (d_r % 128).astype(np.float32)
        row_of_dst_all.append(None)

        xT = np.zeros((F_IN, SHP), np.float32)
        xT[:, :SH] = x[r*SH:(r+1)*SH].T
        in_maps.append({"xT": xT, "osrc": osrc, "odst": odst, "dloc": dloc})

    def aug(W, a_s, a_d):
        W = np.asarray(W, np.float32)
        out = np.zeros((128, 130), np.float32)
        out[:W.shape[0], :128] = W
        out[:W.shape[0], 128] = W @ np.asarray(a_s, np.float32)
        out[:W.shape[0], 129] = W @ np.asarray(a_d, np.float32)
        return out

    shared = {
        "waug0": aug(W0, as0, ad0), "waug1": aug(W1, as1, ad1),
        "waug2": aug(W2, as2, ad2),
        "wout": np.asarray(Wout, np.float32),
        "bb0": np.tile(np.asarray(b0, np.float32), (128, 1)),
        "bb1": np.tile(np.asarray(b1, np.float32), (128, 1)),
        "bb2": np.tile(np.asarray(b2, np.float32), (128, 1)),
        "bbo": np.tile(np.asarray(bout, np.float32), (128, 1)),
        "iota": np.tile(np.arange(128, dtype=np.float32), (128, 1)),
        "ident": np.eye(128, dtype=np.float32),
        "padneg": np.concatenate([np.zeros(106, np.float32),
                                  np.full(22, -1e30, np.float32)])[:, None],
    }
    for m in in_maps:
        m.update(shared)
    return smax, in_maps


def kernel(**inputs):
    global LAST
    smax, in_maps = _prep(**inputs)
    if smax not in _CACHE:
        _CACHE[smax] = _build(smax)
    nc = _CACHE[smax]
    res = run_bass_kernel_spmd(nc, in_maps, list(range(NC_)))
    LAST = res
    return np.concatenate([res.results[r]["out"][:SH] for r in range(NC_)], axis=0)



# revision 9
# speedup vs baseline: 1.7839x; 1.7839x over previous
"""3-layer GAT (graph attention) on 8 trn2 NeuronCores.

Strategy: shard destination nodes across cores (6250/core, padded to
6272 rows). Per layer: each core computes its shard of an augmented
table  [h | es | ed | 1 | pad]  (h = y@W, es/ed = per-node attention
terms) in fp16 via TensorE, AllGathers the 50176x132 fp16 table, then
processes its edges (sorted by dst, per-tile subtile schedule shared
across cores):

- per subtile, ONE [128,1]-offset indirect DMA gathers the 128 src
  rows (264B each); es[src] rides along in column 128.
- ed[dst] is NEVER gathered: an ed row (per-node dst attention term) is
  computed directly as (W a_d)^T @ ytc via 1-column matmuls, broadcast
  to all partitions with a ones-matmul, and expanded per-edge with the
  same one-hot(dst) mask used by the scatter (mask*row, reduce-X).
- a one-hot(dst)*exp(logit) fp16 matrix is built per tile in two
  broadcast VectorE ops, and fp16 TensorE matmuls accumulate weighted
  feature sums + softmax denominator per 128-dst tile in f32 PSUM.
- pad slots use dloc=200: the one-hot row is all-zero, so they
  contribute nothing regardless of what the pad gather returns.
"""
import sys
sys.path.insert(0, "/opt/trn_rl_repo")
import numpy as np
import concourse.bass as bass
import concourse.mybir as mybir
import concourse.tile as tile
from concourse.bass_utils import run_bass_kernel_spmd

N = 50000
E = 600000
F_IN, HID, F_OUT = 64, 128, 64
NC_ = 8
SH = 6250          # real nodes per shard
SHP = 6272         # padded shard rows (49 * 128)
NT = NC_ * SHP     # table rows
NTILE = SHP // 128 # 49 dst tiles per core
TW = 132           # table row width: h(128), es, ed, one, pad
ECH = 512          # ed-row chunk (PSUM bank width in f32)
NECH = SHP // ECH  # 12.25 -> handled with a remainder chunk
NEG_ATT = 0.2
NEG_ACT = 0.01
DUMMY = 0          # table row gathered by pad slots (any valid row)
PADLOC = 200.0     # dloc for pad slots: no one-hot match -> zero contribution

f32 = mybir.dt.float32
f16 = mybir.dt.float16
i32 = mybir.dt.int32


def _hoist_waits(nc):
    """walrus on this toolchain allows only ONE sync-wait slot per
    instruction; move extras onto preceding same-engine NoOps."""
    for fn in nc.m.functions:
        for blk in fn.blocks:
            new_insts = []
            for inst in blk.instructions:
                si = getattr(inst, "sync_info", None)
                waits = list(si.on_wait) if si is not None and si.on_wait else []
                if len(waits) > 1:
                    keep, extra = waits[:1], waits[1:]
                    while extra:
                        chunk, extra = extra[:1], extra[1:]
                        nop = mybir.InstNoOp(
                            name=nc.get_next_instruction_name(),
                            engine=inst.engine, bass_nofuse=True)
                        nop.sync_info = mybir.SyncInfo(on_wait=chunk, on_update=[])
                        new_insts.append(nop)
                    inst.sync_info = mybir.SyncInfo(
                        on_wait=keep,
                        on_update=list(si.on_update) if si.on_update else [])
                new_insts.append(inst)
            blk.instructions = new_insts


def _build(cd):
    cd = list(cd)
    kmax = max(cd)
    nsub = sum(cd)
    starts = np.zeros(NTILE, np.int64)
    starts[1:] = np.cumsum(cd)[:-1]
    # ed-row chunks over the SHP node axis
    chunks = []
    off = 0
    while off < SHP:
        w = min(ECH, SHP - off)
        chunks.append((off, w))
        off += w

    nc = bass.Bass()
    xT = nc.declare_dram_parameter("xT", [F_IN, SHP], f32, isOutput=False)
    osrc = nc.declare_dram_parameter("osrc", [128, nsub], i32, isOutput=False)
    dloc = nc.declare_dram_parameter("dloc", [128, nsub], f16, isOutput=False)
    waug = [nc.declare_dram_parameter(f"waug{l}", [128, 130], f32, isOutput=False)
            for l in range(3)]
    wout = nc.declare_dram_parameter("wout", [128, F_OUT], f32, isOutput=False)
    bb = [nc.declare_dram_parameter(f"bb{l}", [128, 128], f32, isOutput=False)
          for l in range(3)]
    bbo = nc.declare_dram_parameter("bbo", [128, F_OUT], f32, isOutput=False)
    iota = nc.declare_dram_parameter("iota", [128, 128], f16, isOutput=False)
    ident = nc.declare_dram_parameter("ident", [128, 128], f32, isOutput=False)
    out_e = nc.declare_dram_parameter("out", [SHP, F_OUT], f32, isOutput=True)

    tbl_in = [nc.dram_tensor(f"tbl_in{l}", [SHP, TW], f16) for l in range(3)]
    tbl = [nc.dram_tensor(f"tbl{l}", [NT, TW], f16, addr_space="Shared")
           for l in range(3)]

    with tile.TileContext(nc) as tc:
        with (
            tc.tile_pool(name="const", bufs=1) as cpool,
            tc.tile_pool(name="stage", bufs=4) as spool,
            tc.tile_pool(name="gat", bufs=3) as gpool,
            tc.tile_pool(name="edf", bufs=3) as efpool,
            tc.tile_pool(name="lgt", bufs=4) as lpool,
            tc.tile_pool(name="bx", bufs=3) as bxpool,
            tc.tile_pool(name="edm", bufs=2) as empool,
            tc.tile_pool(name="epi", bufs=4) as tpool,
            tc.tile_pool(name="ps_h", bufs=2, space="PSUM") as ps_h,
            tc.tile_pool(name="ps_nm", bufs=2, space="PSUM") as ps_nm,
            tc.tile_pool(name="ps_t", bufs=1, space="PSUM") as ps_t,
            tc.tile_pool(name="ps_e", bufs=1, space="PSUM") as ps_e,
            tc.tile_pool(name="ps_r", bufs=1, space="PSUM") as ps_r,
        ):
            # ---- resident constants / inputs
            osrc_sb = cpool.tile([128, nsub], i32)
            dloc_sb = cpool.tile([128, nsub], f16)
            nc.sync.dma_start(out=osrc_sb[:], in_=osrc[:])
            nc.sync.dma_start(out=dloc_sb[:], in_=dloc[:])
            waug_sb = [cpool.tile([128, 130], f32, name=f"waug_sb{l}") for l in range(3)]
            for l in range(3):
                nc.sync.dma_start(out=waug_sb[l][:], in_=waug[l][:])
            wout_sb = cpool.tile([128, F_OUT], f32)
            nc.sync.dma_start(out=wout_sb[:], in_=wout[:])
            bb_sb = [cpool.tile([128, 128], f32, name=f"bb_sb{l}") for l in range(3)]
            for l in range(3):
                nc.sync.dma_start(out=bb_sb[l][:], in_=bb[l][:])
            bbo_sb = cpool.tile([128, F_OUT], f32)
            nc.sync.dma_start(out=bbo_sb[:], in_=bbo[:])
            iota_sb = cpool.tile([128, 128], f16)
            nc.sync.dma_start(out=iota_sb[:], in_=iota[:])
            id_sb = cpool.tile([128, 128], f32)
            nc.sync.dma_start(out=id_sb[:], in_=ident[:])
            ones1_sb = cpool.tile([1, 128], f16)
            nc.vector.memset(ones1_sb[:], 1.0)
            edrep_sb = cpool.tile([128, SHP], f16)   # ed per node, all partitions

            # YT double buffer: layer input, feat x nodes (feat on partitions)
            yt_sb = [cpool.tile([128, SHP], f32, name=f"yt{i}") for i in range(2)]
            nc.sync.dma_start(out=yt_sb[0][:F_IN, :], in_=xT[:])

            for l in range(3):
                fin = F_IN if l == 0 else HID
                ytc = yt_sb[l % 2]
                ytn = yt_sb[(l + 1) % 2]

                # ---- phase 1a: ed row (per local node) = (W a_d)^T @ ytc,
                # broadcast to all 128 partitions via ones-matmul.
                for (coff, cw) in chunks:
                    efp = ps_e.tile([1, ECH], f32, tag="efp")
                    nc.tensor.matmul(efp[:, :cw],
                                     lhsT=waug_sb[l][:fin, 129:130],
                                     rhs=ytc[:fin, coff:coff+cw],
                                     start=True, stop=True)
                    efs = efpool.tile([1, ECH], f16, tag="efs")
                    nc.vector.tensor_copy(out=efs[:, :cw], in_=efp[:, :cw])
                    erp = ps_r.tile([128, ECH], f32, tag="erp")
                    nc.tensor.matmul(erp[:, :cw], lhsT=ones1_sb[:],
                                     rhs=efs[:, :cw], start=True, stop=True)
                    nc.vector.tensor_copy(out=edrep_sb[:, coff:coff+cw],
                                          in_=erp[:, :cw])

                # ---- phase 1b: shard of augmented table = yT' @ [W|ws|wd]
                for t in range(NTILE):
                    hps = ps_h.tile([128, 130], f32, tag="hps")
                    nc.tensor.matmul(hps[:], lhsT=ytc[:fin, t*128:(t+1)*128],
                                     rhs=waug_sb[l][:fin, :], start=True, stop=True)
                    stg = spool.tile([128, TW], f16, tag="stg")
                    nc.vector.tensor_copy(out=stg[:, 0:130], in_=hps[:])
                    nc.vector.memset(stg[:, 130:131], 1.0)
                    nc.vector.memset(stg[:, 131:132], 0.0)
                    nc.sync.dma_start(out=tbl_in[l][t*128:(t+1)*128, :], in_=stg[:])

                # ---- phase 2: allgather table
                nc.gpsimd.collective_compute(
                    "AllGather", mybir.AluOpType.bypass,
                    replica_groups=[list(range(NC_))],
                    ins=[tbl_in[l][:]], outs=[tbl[l][:]])

                # ---- phase 3: edges, per dst tile
                for d in range(NTILE):
                    k = cd[d]
                    j0 = int(starts[d])
                    g = gpool.tile([128, kmax * TW], f16, tag="G")
                    for s in range(k):
                        nc.gpsimd.indirect_dma_start(
                            out=g[:, s*TW:(s+1)*TW], out_offset=None,
                            in_=tbl[l][:],
                            in_offset=bass.IndirectOffsetOnAxis(
                                ap=osrc_sb[:, j0+s:j0+s+1], axis=0))
                    # one-hot(dst) for all k subtiles in one broadcast op
                    bxt = bxpool.tile([128, kmax * 128], f16, tag="bxt")
                    b3 = bxt[:, 0:k*128].rearrange("p (k c) -> p k c", k=k)
                    nc.vector.tensor_tensor(
                        out=b3,
                        in0=iota_sb[:].unsqueeze(1).broadcast_to((128, k, 128)),
                        in1=dloc_sb[:, j0:j0+k].unsqueeze(2).broadcast_to(
                            (128, k, 128)),
                        op=mybir.AluOpType.is_equal)
                    # ed per edge = reduce(one-hot * ed_row_tile)
                    edm = empool.tile([128, kmax * 128], f16, tag="edm")
                    e3 = edm[:, 0:k*128].rearrange("p (k c) -> p k c", k=k)
                    nc.vector.tensor_tensor(
                        out=e3, in0=b3,
                        in1=edrep_sb[:, d*128:(d+1)*128].unsqueeze(1)
                            .broadcast_to((128, k, 128)),
                        op=mybir.AluOpType.mult)
                    ede = lpool.tile([128, kmax], f32, tag="ede")
                    nc.vector.tensor_reduce(out=ede[:, :k], in_=e3,
                                            axis=mybir.AxisListType.X,
                                            op=mybir.AluOpType.add)
                    # logits -> exp  (es gathered in column 128 of each row)
                    esb = lpool.tile([128, kmax], f32, tag="esb")
                    etm = lpool.tile([128, kmax], f32, tag="etm")
                    exb = lpool.tile([128, kmax], f16, tag="exb")
                    nc.vector.tensor_tensor(
                        out=esb[:, :k], in0=g[:, 128:k*TW:TW],
                        in1=ede[:, :k], op=mybir.AluOpType.add)
                    nc.vector.tensor_scalar_mul(etm[:, :k], esb[:, :k], NEG_ATT)
                    nc.vector.tensor_tensor(out=esb[:, :k], in0=esb[:, :k],
                                            in1=etm[:, :k],
                                            op=mybir.AluOpType.max)
                    nc.scalar.activation(exb[:, :k], esb[:, :k],
                                         mybir.ActivationFunctionType.Exp)
                    # bx = one-hot * exp(e), all k subtiles in one op
                    nc.vector.tensor_tensor(
                        out=b3, in0=b3,
                        in1=exb[:, 0:k].unsqueeze(2).broadcast_to((128, k, 128)),
                        op=mybir.AluOpType.mult)
                    nmps = ps_nm.tile([128, 131], f32, tag="nm")
                    for s in range(k):
                        nc.tensor.matmul(nmps[:], lhsT=bxt[:, s*128:(s+1)*128],
                                         rhs=g[:, s*TW:s*TW+131],
                                         start=(s == 0), stop=(s == k - 1))
                    # ---- epilogue: y = numer/denom + b, activation, transpose
                    dn = tpool.tile([128, 1], f32, tag="dn")
                    nc.vector.tensor_scalar_add(dn[:], nmps[:, 130:131], 1e-16)
                    rec = tpool.tile([128, 1], f32, tag="rec")
                    nc.vector.reciprocal(rec[:], dn[:])
                    y = tpool.tile([128, 128], f32, tag="y")
                    nc.vector.tensor_scalar(
                        out=y[:], in0=nmps[:, 0:128], scalar1=rec[:],
                        scalar2=None, op0=mybir.AluOpType.mult)
                    nc.vector.tensor_tensor(out=y[:], in0=y[:], in1=bb_sb[l][:],
                                            op=mybir.AluOpType.add)
                    y2 = tpool.tile([128, 128], f32, tag="y2")
                    nc.vector.tensor_scalar_mul(y2[:], y[:], NEG_ACT)
                    nc.vector.tensor_tensor(out=y[:], in0=y[:], in1=y2[:],
                                            op=mybir.AluOpType.max)
                    tps = ps_t.tile([128, 128], f32, tag="tps")
                    nc.tensor.transpose(tps[:], y[:], id_sb[:])
                    if l < 2:
                        nc.vector.tensor_copy(out=ytn[:, d*128:(d+1)*128], in_=tps[:])
                    else:
                        y3t = tpool.tile([128, 128], f32, tag="y3t")
                        nc.vector.tensor_copy(out=y3t[:], in_=tps[:])
                        ops = ps_t.tile([128, F_OUT], f32, tag="ops")
                        nc.tensor.matmul(ops[:], lhsT=y3t[:], rhs=wout_sb[:],
                                         start=True, stop=True)
                        ot = tpool.tile([128, F_OUT], f32, tag="ot")
                        nc.vector.tensor_tensor(out=ot[:], in0=ops[:], in1=bbo_sb[:],
                                                op=mybir.AluOpType.add)
                        nc.sync.dma_start(out=out_e[d*128:(d+1)*128, :], in_=ot[:])
    _hoist_waits(nc)
    return nc


_CACHE = {}
LAST = None  # last BassKernelResults (for test harness introspection)


def _prep(x, edge_index, W0, as0, ad0, b0, W1, as1, ad1, b1,
          W2, as2, ad2, b2, Wout, bout):
    x = np.asarray(x, np.float32)
    ei = np.asarray(edge_index)
    src = np.concatenate([ei[0], np.arange(N, dtype=np.int64)]).astype(np.int64)
    dst = np.concatenate([ei[1], np.arange(N, dtype=np.int64)]).astype(np.int64)

    core = dst // SH
    row_of_src = (src // SH) * SHP + (src % SH)   # global table row of src

    per_core = []
    cnts = np.zeros((NC_, NTILE), np.int64)
    for r in range(NC_):
        m = core == r
        s_r = row_of_src[m]
        d_r = dst[m] - r * SH           # local dst 0..6249
        o = np.argsort(d_r, kind="stable")
        s_r, d_r = s_r[o], d_r[o]
        t_r = d_r // 128                # dst tile
        cnt = np.bincount(t_r, minlength=NTILE)
        cnts[r] = cnt
        per_core.append((s_r, d_r, t_r, cnt))

    # per-tile subtile counts, shared across cores (SPMD program uniformity)
    cd = np.maximum(1, np.ceil(cnts.max(axis=0) / 128.0).astype(np.int64))
    starts = np.zeros(NTILE, np.int64)
    starts[1:] = np.cumsum(cd)[:-1]
    nsub = int(cd.sum())

    in_maps = []
    for r in range(NC_):
        s_r, d_r, t_r, cnt = per_core[r]
        osrc = np.full((128, nsub), DUMMY, np.int32)
        dloc = np.full((128, nsub), PADLOC, np.float16)
        tstart = np.zeros(NTILE, np.int64)
        tstart[1:] = np.cumsum(cnt)[:-1]
        kk = np.arange(len(d_r)) - tstart[t_r]     # rank within dst tile
        sub = starts[t_r] + (kk // 128)            # subtile slot
        lane = kk % 128
        osrc[lane, sub] = s_r
        dloc[lane, sub] = (d_r % 128).astype(np.float16)

        xT = np.zeros((F_IN, SHP), np.float32)
        xT[:, :SH] = x[r*SH:(r+1)*SH].T
        in_maps.append({"xT": xT, "osrc": osrc, "dloc": dloc})

    def aug(W, a_s, a_d):
        W = np.asarray(W, np.float32)
        out = np.zeros((128, 130), np.float32)
        out[:W.shape[0], :128] = W
        out[:W.shape[0], 128] = W @ np.asarray(a_s, np.float32)
        out[:W.shape[0], 129] = W @ np.asarray(a_d, np.float32)
        return out

    shared = {
        "waug0": aug(W0, as0, ad0), "waug1": aug(W1, as1, ad1),
        "waug2": aug(W2, as2, ad2),
        "wout": np.asarray(Wout, np.float32),
        "bb0": np.tile(np.asarray(b0, np.float32), (128, 1)),
        "bb1": np.tile(np.asarray(b1, np.float32), (128, 1)),
        "bb2": np.tile(np.asarray(b2, np.float32), (128, 1)),
        "bbo": np.tile(np.asarray(bout, np.float32), (128, 1)),
        "iota": np.tile(np.arange(128, dtype=np.float16), (128, 1)),
        "ident": np.eye(128, dtype=np.float32),
    }
    for m in in_maps:
        m.update(shared)
    return tuple(cd.tolist()), in_maps


def kernel(**inputs):
    global LAST
    cd, in_maps = _prep(**inputs)
    if cd not in _CACHE:
        _CACHE[cd] = _build(cd)
    nc = _CACHE[cd]
    res = run_bass_kernel_spmd(nc, in_maps, list(range(NC_)))
    LAST = res
    return np.concatenate([res.results[r]["out"][:SH] for r in range(NC_)], axis=0)
